# revision 10
# baseline (speedup 1.0000x reference)
"""MoE experts (32 experts, top-2, SwiGLU MLP) on 8 trn2 NeuronCores.

Expert-parallel sharding: core c owns experts [4c, 4c+4). Routing metadata
(Switch-style positions / per-expert slot lists) is computed on host from
top_k_indices; each core receives its 4 experts' weights (pre-transposed to
matmul layout) plus the dispatched token activations, runs the grouped
SwiGLU MLP + routing-weight scaling on device, and returns per-slot
outputs. Host scatters per-slot outputs back to (token, k) and sums over
the top-k axis and cores (the expert-parallel combine/unshard).
"""

import sys
import types

import numpy as np

# Model dims (hardcoded per problem spec nn_MoEExperts_27109833572673)
T, TOPK, E, H, I = 4096, 2, 32, 512, 1024
CAP = 2 * (T * TOPK) // E  # 512
NCORES = 8
EPC = E // NCORES  # experts per core = 4
HT = H // 128  # 4 h-tiles
IT = I // 128  # 8 i-tiles

# How matmuls run: "f32" (exact, 4 cyc/row), "f32r" (1 cyc/row), "bf16"
MM_DTYPE = "f32r"

LAST_RESULTS = None  # BassKernelResults of the most recent device run


def _ensure_profile_hook():
    """Register the NTFF profile hook if the env lacks antenv.axon_hooks.

    Only needed when tracing (BASS_TRACE=1 / trace=True); safe no-op
    otherwise. Mirrors trn_agent_boot.trn_boot step 6.
    """
    try:
        if "antenv.axon_hooks" in sys.modules:
            return
        import antenv

        mod = types.ModuleType("antenv.axon_hooks")
        state = {"hook": None}
        mod.set_axon_ntff_profile_hook = lambda h: state.__setitem__("hook", h)
        mod.get_axon_ntff_profile_hook = lambda: state["hook"]
        sys.modules["antenv.axon_hooks"] = mod
        antenv.axon_hooks = mod
        try:
            from trn_agent_boot.trn_boot import _ntff_profile_via_ctypes

            mod.set_axon_ntff_profile_hook(
                _ntff_profile_via_ctypes("/opt/axon/libaxon_pjrt.so")
            )
        except Exception:
            pass
    except Exception:
        pass


def _routing(top_k_indices, top_k_weights):
    """Per-expert slot lists (ascending flat order == Switch dispatch pos),
    clipped at CAP exactly like the reference's capacity drop."""
    e_flat = np.asarray(top_k_indices).reshape(-1).astype(np.int32)
    w_flat = np.asarray(top_k_weights).reshape(-1).astype(np.float32)
    tok = np.arange(T * TOPK, dtype=np.int32) // TOPK
    order = np.argsort(e_flat, kind="stable")
    sorted_e = e_flat[order]
    starts = np.searchsorted(sorted_e, np.arange(E + 1))
    slots_per_e = [order[starts[e] : starts[e + 1]][:CAP] for e in range(E)]
    return e_flat, w_flat, tok, slots_per_e


_prog_cache = {}


def _build_program(m_pad):
    """One SPMD program: per-core grouped SwiGLU MLP over EPC experts with
    m_pad padded slots each."""
    import concourse.bacc as bacc
    import concourse.mybir as mybir
    from concourse.tile import TileContext

    f32 = mybir.dt.float32
    f32r = mybir.dt.float32r
    mmdt = {"f32": f32, "f32r": f32r,
            "bf16": mybir.dt.bfloat16}[MM_DTYPE]
    slots = EPC * m_pad
    mt = m_pad // 128

    nc = bacc.Bacc("TRN2", target_bir_lowering=False, debug=False,
                   num_devices=NCORES)
    # Host lays every input out so each device DMA is one plain [128, X]
    # copy: xdT[j, p, ht*m_pad + s], w1t[j, p, ht*2I + o],
    # w2t[j, p, it*H + h], y[j, p, m*H + h].
    xdT_d = nc.declare_dram_parameter("xdT", [EPC, 128, HT * m_pad], mmdt,
                                      isOutput=False)
    w1t_d = nc.declare_dram_parameter("w1t", [EPC, 128, HT * 2 * I], mmdt,
                                      isOutput=False)
    w2t_d = nc.declare_dram_parameter("w2t", [EPC, 128, IT * H], mmdt,
                                      isOutput=False)
    wsc_d = nc.declare_dram_parameter("wsc", [128, slots // 128], f32,
                                      isOutput=False)
    y_d = nc.declare_dram_parameter("y", [EPC, 128, mt * H], f32,
                                    isOutput=True)

    with TileContext(nc) as tc:
        with (
            tc.tile_pool(name="xd", bufs=2) as xdp,
            tc.tile_pool(name="w1", bufs=2) as w1p,
            tc.tile_pool(name="w2", bufs=2) as w2p,
            tc.tile_pool(name="act", bufs=2) as actp,
            tc.tile_pool(name="ps1", bufs=2, space="PSUM") as ps1p,
            tc.tile_pool(name="ps2", bufs=2, space="PSUM") as ps2p,
            tc.tile_pool(name="outp", bufs=2) as outp,
            tc.tile_pool(name="misc", bufs=1) as miscp,
        ):
            wsc_t = miscp.tile([128, slots // 128], f32, tag="wsc")
            nc.sync.dma_start(out=wsc_t[:], in_=wsc_d[:])

            for j in range(EPC):
                # per-ht tiles so the first matmul only waits on the first
                # ~1.2MB instead of the whole 4.9MB expert payload
                xd = []
                w1 = []
                for ht in range(HT):
                    xt = xdp.tile([128, m_pad], mmdt, tag=f"xd{ht}",
                                  name=f"xd{ht}")
                    nc.sync.dma_start(
                        out=xt[:],
                        in_=xdT_d[j, :, ht * m_pad : (ht + 1) * m_pad])
                    xd.append(xt)
                    wt = w1p.tile([128, 2 * I], mmdt, tag=f"w1_{ht}",
                                  name=f"w1_{ht}")
                    nc.sync.dma_start(
                        out=wt[:],
                        in_=w1t_d[j, :, ht * 2 * I : (ht + 1) * 2 * I])
                    w1.append(wt)
                w2 = w2p.tile([128, IT * H], mmdt, tag="w2", name="w2")

                # mm1: out1^T[o, s] = sum_h W1[o, h] * xd[s, h], per o-tile,
                # gate rows are o in [0, I), up rows are o in [I, 2I)
                acts = []
                for it in range(IT):
                    pg = ps1p.tile([128, m_pad], f32, tag="pg", name="pg")
                    pu = ps1p.tile([128, m_pad], f32, tag="pu", name="pu")
                    for ht in range(HT):
                        o0 = it * 128
                        nc.tensor.matmul(
                            pg[:], w1[ht][:, o0 : o0 + 128], xd[ht][:],
                            start=(ht == 0), stop=(ht == HT - 1))
                    for ht in range(HT):
                        o0 = (IT + it) * 128
                        nc.tensor.matmul(
                            pu[:], w1[ht][:, o0 : o0 + 128], xd[ht][:],
                            start=(ht == 0), stop=(ht == HT - 1))
                    sg = actp.tile([128, m_pad], f32, tag="sg", name="sg")
                    nc.scalar.activation(
                        sg[:], pg[:], mybir.ActivationFunctionType.Silu)
                    a = actp.tile([128, m_pad], mmdt, tag=f"a{it}",
                                  name=f"a{it}")
                    nc.vector.tensor_mul(a[:], sg[:], pu[:])
                    acts.append(a)
                    if it == 0:
                        # issue w2 load only after mm1 is underway so it
                        # doesn't steal DMA bandwidth from the critical path
                        nc.sync.dma_start(out=w2[:], in_=w2t_d[j])

                # mm2: y[s, h] = sum_i act[s, i] * W2[h, i], slots on PSUM
                # partitions; then scale rows by routing weight and store.
                ot = outp.tile([128, mt * H], f32, tag="ot", name="ot")
                for m in range(mt):
                    ps2 = ps2p.tile([128, H], f32, tag="ps2", name="ps2")
                    for it in range(IT):
                        nc.tensor.matmul(
                            ps2[:], acts[it][:, m * 128 : (m + 1) * 128],
                            w2[:, it * H : (it + 1) * H],
                            start=(it == 0), stop=(it == IT - 1))
                    col = j * mt + m
                    nc.vector.tensor_scalar_mul(
                        ot[:, m * H : (m + 1) * H], ps2[:],
                        wsc_t[:, col : col + 1])
                # store on the ACT HWDGE ring so it never queues ahead of
                # the next expert's weight loads on the SP ring
                nc.scalar.dma_start(out=y_d[j], in_=ot[:])

    nc.finalize()
    return nc


def kernel(hidden_states, top_k_indices, top_k_weights, gate_up_proj,
           down_proj):
    global LAST_RESULTS
    _ensure_profile_hook()
    from concourse.bass_utils import run_bass_kernel_spmd

    hs = np.ascontiguousarray(np.asarray(hidden_states, dtype=np.float32))
    gup = np.asarray(gate_up_proj, dtype=np.float32)
    dwn = np.asarray(down_proj, dtype=np.float32)

    e_flat, w_flat, tok, slots_per_e = _routing(top_k_indices, top_k_weights)
    counts = [len(s) for s in slots_per_e]
    m_pad = max(128, ((max(counts) + 127) // 128) * 128)
    m_pad = min(m_pad, CAP)
    slots = EPC * m_pad
    mt = m_pad // 128

    if m_pad not in _prog_cache:
        _prog_cache[m_pad] = _build_program(m_pad)
    nc = _prog_cache[m_pad]

    in_maps = []
    for c in range(NCORES):
        exps = range(c * EPC, (c + 1) * EPC)
        # dispatched tokens, transposed: xdT[j, p, ht*m_pad + s]
        xd = np.zeros((EPC, m_pad, H), np.float32)
        wsc = np.zeros((EPC, m_pad), np.float32)
        for jj, e in enumerate(exps):
            sl = slots_per_e[e]
            xd[jj, : len(sl)] = hs[tok[sl]]
            wsc[jj, : len(sl)] = w_flat[sl]
        # [j, s, ht, p] -> [j, p, ht, s]
        xdT = np.ascontiguousarray(
            xd.reshape(EPC, m_pad, HT, 128).transpose(0, 3, 2, 1)
        ).reshape(EPC, 128, HT * m_pad)
        # w1t[j, p, ht*2I + o] = gate_up[e_j, o, ht*128 + p]
        w1t = np.ascontiguousarray(
            gup[c * EPC : (c + 1) * EPC]
            .reshape(EPC, 2 * I, HT, 128).transpose(0, 3, 2, 1)
        ).reshape(EPC, 128, HT * 2 * I)
        # w2t[j, p, it*H + h] = down[e_j, h, it*128 + p]
        w2t = np.ascontiguousarray(
            dwn[c * EPC : (c + 1) * EPC]
            .reshape(EPC, H, IT, 128).transpose(0, 3, 2, 1)
        ).reshape(EPC, 128, IT * H)
        # wsc[p, j*mt + m] = w for slot (j, m*128 + p)
        wsc_m = np.ascontiguousarray(
            wsc.reshape(EPC * mt, 128).T)
        in_maps.append({"xdT": xdT, "w1t": w1t, "w2t": w2t, "wsc": wsc_m})

    res = run_bass_kernel_spmd(nc, in_maps, list(range(NCORES)))
    LAST_RESULTS = res

    # Combine: scatter per-slot outputs back to flat (token, k) slots and
    # reduce over the top-k axis and cores.
    y_tk = np.zeros((T * TOPK, H), np.float32)
    for c in range(NCORES):
        # y[j, p, m*H + h] -> [j, m*128 + p, h]
        yc = (res.results[c]["y"].reshape(EPC, 128, mt, H)
              .transpose(0, 2, 1, 3).reshape(EPC, m_pad, H))
        for jj, e in enumerate(range(c * EPC, (c + 1) * EPC)):
            sl = slots_per_e[e]
            y_tk[sl] = yc[jj, : len(sl)]
    return y_tk.reshape(T, TOPK, H).sum(axis=1)
